# revision 6
# baseline (speedup 1.0000x reference)
"""Trainium2 Bass kernel: 16-head attention (SEQ=4096, D_MODEL=1024, D_K=64).

Sharding: tensor-parallel over heads. 2 heads per core x 8 cores.
W_O is row-sharded; each core returns a partial [S, D] output projection,
summed on the host (the all-reduce of the output projection).

Per-core dataflow (all matmuls fp32r = FP22-truncated full-rate):
  qT/kT [64,S] per head via projections on transposed inputs (QT/KT in DRAM)
  v natural [S,64] per head (direct matmul from VT chunks), augmented with a
    ones column so attention@V also yields softmax row-sums.
  Natural pass (scores [q,s], fp16 operands, 2 heads packed concurrently on
    PE row groups): row maxes via one wide DVE reduce per [128,2,512] unit.
  maxes transposed via a tiny matmul against -I, written into row 64 of the
    augmented qT tiles.
  Fine pass (transposed scores [s,q]): K=65 matmul computes scores^T - max(q)
    directly, two 128-chunk banks per PSUM pair; one ScalarE exp per
    [128,1024] pair -> E fp16; [v|1] @ E accumulates att@V and row sums.
  Normalize via reciprocal + ones-replication matmul; W_O on normalized
    concatenated heads; partial out DMA'd back.

Pipelining: tile t's fine/av loop streams tile t+1's q-projection and
natural/max pass between its steps; PSUM banks: av 2 + fine-pairs 4 + nat 2.
"""

import os
import sys

import numpy as np

for _p in (
    "/root/.axon_site",
    "/root/.axon_site/_ro/trn_rl_repo",
    "/root/.axon_site/_ro/pypackages",
    "/opt/trn_rl_repo",
    "/opt/pypackages",
):
    if os.path.isdir(_p) and _p not in sys.path:
        sys.path.append(_p)

D = 1024
NHEADS = 16
DK = 64
NCORES = 8
S_FULL = 4096

_cache = {}
LAST_RESULT = None  # BassKernelResults of the most recent run (for test harness)


def _build(S):
    import concourse.bass as bass  # noqa: F401
    import concourse.tile as tile
    from concourse import bacc, mybir
    from concourse.masks import make_identity
    from contextlib import ExitStack

    f32 = mybir.dt.float32
    f32r = mybir.dt.float32r
    fp16 = mybir.dt.float16
    X = mybir.AxisListType.X
    Exp = mybir.ActivationFunctionType.Exp

    NT = S // 512   # 512-wide q tiles
    NCH = S // 128  # 128-wide s chunks
    NP = NCH // 2   # pairs of s chunks per tile
    ND = D // 128   # contraction chunks
    NSH = S // 512  # 512-wide s slices for the natural pass

    nc = bacc.Bacc(
        "TRN2",
        target_bir_lowering=False,
        debug=False,
        num_devices=NCORES,
    )
    qt = nc.dram_tensor("qt", [D, S], f32r, kind="ExternalInput")
    kt = nc.dram_tensor("kt", [D, S], f32r, kind="ExternalInput")
    vt = nc.dram_tensor("vt", [D, S], f32r, kind="ExternalInput")
    wq = nc.dram_tensor("wq", [D, 128], f32r, kind="ExternalInput")
    wk = nc.dram_tensor("wk", [D, 128], f32r, kind="ExternalInput")
    wv = nc.dram_tensor("wv", [D, 128], f32r, kind="ExternalInput")
    wo = nc.dram_tensor("wo", [128, D], f32r, kind="ExternalInput")
    out = nc.dram_tensor("out", [S, D], f32, kind="ExternalOutput")

    with tile.TileContext(nc) as tc, ExitStack() as ctx:
        consts = ctx.enter_context(tc.tile_pool(name="consts", bufs=1))
        big = ctx.enter_context(tc.tile_pool(name="big", bufs=1))
        ldpool = ctx.enter_context(tc.tile_pool(name="ld", bufs=3))
        epool = ctx.enter_context(tc.tile_pool(name="e", bufs=3))
        smalls = ctx.enter_context(tc.tile_pool(name="smalls", bufs=4))
        outp = ctx.enter_context(tc.tile_pool(name="outp", bufs=2))
        ps_nat = ctx.enter_context(tc.tile_pool(name="ps_nat", bufs=1, space="PSUM"))
        ps_av = ctx.enter_context(tc.tile_pool(name="ps_av", bufs=1, space="PSUM"))
        ps_pf = ctx.enter_context(tc.tile_pool(name="ps_pf", bufs=2, space="PSUM"))

        def pfine():
            # rotating 2-bank [128, 1024] fp32 tile: fine pairs, W_O, q-proj,
            # k-proj, v-proj, maxT, reps all share this rotation
            return ps_pf.tile([128, 1024], f32, tag="pf", name="pf")

        def pnat():
            return ps_nat.tile([128, 1024], f32, tag="pn", name="pn")

        # constants
        ident_f = consts.tile([128, 128], f32)
        make_identity(nc, ident_f)
        identn = consts.tile([128, 128], f32r)  # -I, rounded for fp32r matmul
        nc.vector.tensor_scalar_mul(identn[:], ident_f[:], -1.0)
        ones64 = consts.tile([1, 64], f32r)
        nc.vector.memset(ones64[:].bitcast(f32), 1.0)

        # weights
        wq_sb = consts.tile([128, ND, 128], f32r)
        nc.sync.dma_start(wq_sb[:], wq.rearrange("(o p) f -> p o f", p=128))
        wk_sb = consts.tile([128, ND, 128], f32r)
        nc.sync.dma_start(wk_sb[:], wk.rearrange("(o p) f -> p o f", p=128))
        wv_sb = consts.tile([128, ND, 128], f32r)
        nc.sync.dma_start(wv_sb[:], wv.rearrange("(o p) f -> p o f", p=128))
        wo_sb = consts.tile([128, D], f32r)
        nc.sync.dma_start(wo_sb[:], wo[:])

        # big SBUF tensors
        qaug = [big.tile([65, S], f32r, tag=f"qaug{h}", name=f"qaug{h}") for h in range(2)]
        kaug = [big.tile([65, S], f32r, tag=f"kaug{h}", name=f"kaug{h}") for h in range(2)]
        v_sb = big.tile([128, NCH, 2, 65], fp16, tag="v", name="v_sb")
        q16 = big.tile([128, S], fp16, tag="q16", name="q16")
        k16 = big.tile([128, S], fp16, tag="k16", name="k16")
        concat = big.tile([128, S], f32r, tag="concat", name="concat")
        for h in range(2):
            nc.vector.memset(kaug[h][64:65, :].bitcast(f32), 1.0)
        nc.vector.memset(v_sb[:, :, :, 64:65], 1.0)

        # per-block running maxes for the natural pass: [128, 2 heads, NSH]
        mx = [smalls.tile([128, 2, NSH], f32, tag=f"mx{b}", name=f"mx{b}")
              for b in range(4)]

        # ---- projection helpers -------------------------------------------
        def proj_qk_tile(src, wsb, dstA, dstB, dst16, t):
            """Project one 512-col slice of q or k; fills aug + fp16 tiles."""
            ps = pfine()
            for d in range(ND):
                lt = ldpool.tile([128, 512], f32r, tag="ld", name="ld")
                nc.sync.dma_start(lt[:], src[d * 128:(d + 1) * 128, t * 512:(t + 1) * 512])
                nc.tensor.matmul(ps[:, 0:512], wsb[:, d, :], lt[:],
                                 start=(d == 0), stop=(d == ND - 1))
            nc.scalar.copy(dstA[0:64, t * 512:(t + 1) * 512], ps[0:64, 0:512])
            nc.scalar.copy(dstB[0:64, t * 512:(t + 1) * 512], ps[64:128, 0:512])
            nc.vector.tensor_copy(dst16[:, t * 512:(t + 1) * 512], ps[:, 0:512])

        def proj_v_tile(t):
            vts = ldpool.tile([128, ND, 512], f32r, tag="vts", name="vts", bufs=2)
            nc.sync.dma_start(
                vts[:], vt[:, t * 512:(t + 1) * 512].rearrange("(o p) s -> p o s", p=128))
            ps = pfine()
            for sc in range(4):
                for d in range(ND):
                    nc.tensor.matmul(ps[:, sc * 128:(sc + 1) * 128],
                                     vts[:, d, sc * 128:(sc + 1) * 128],
                                     wv_sb[:, d, :],
                                     start=(d == 0), stop=(d == ND - 1),
                                     skip_group_check=True)
            for sc in range(4):
                c = t * 4 + sc
                nc.vector.tensor_copy(
                    v_sb[:, c, :, 0:64],
                    ps[:, sc * 128:(sc + 1) * 128].rearrange("p (h f) -> p h f", h=2))

        # ---- natural pass units -------------------------------------------
        def nat_unit(b, sh):
            """Natural scores for q-block b (global 128-block idx), s-slice sh.
            Both heads packed concurrently on PE row groups (0,0)/(64,0),
            one wide DVE max-reduce over both heads' banks."""
            qsl = slice(b * 128, (b + 1) * 128)
            ssl = slice(sh * 512, (sh + 1) * 512)
            pn = pnat()
            nc.tensor.matmul(pn[:, 0:512], q16[0:64, qsl], k16[0:64, ssl],
                             start=True, stop=True)
            nc.tensor.matmul(pn[:, 512:1024], q16[64:128, qsl], k16[64:128, ssl],
                             start=True, stop=True)
            nc.vector.reduce_max(mx[b % 4][:, :, sh:sh + 1],
                                 pn[:].rearrange("p (h s) -> p h s", h=2), axis=X)

        def nat_finish(b):
            """Final per-head max for q-block b -> -max into qaug row 64."""
            qsl = slice(b * 128, (b + 1) * 128)
            m2 = smalls.tile([128, 2, 1], f32r, tag="m2", name="m2")
            nc.vector.reduce_max(m2[:], mx[b % 4][:], axis=X)
            pt = pfine()
            for h in range(2):
                nc.tensor.matmul(pt[0:1, h * 128:(h + 1) * 128],
                                 m2[:, h, :], identn[:],
                                 start=True, stop=True, skip_group_check=True)
            for h in range(2):
                nc.vector.tensor_copy(qaug[h][64:65, qsl],
                                      pt[0:1, h * 128:(h + 1) * 128])

        # ---- prologue -----------------------------------------------------
        # q proj for tile 0 first (small), then k streamed with tile-0 nat
        # units right behind each 512-slice. v-projection is deferred into
        # tile 0's streamed extras so its DMA overlaps attention compute.
        proj_qk_tile(qt, wq_sb, qaug[0], qaug[1], q16, 0)
        for sh in range(NSH):
            proj_qk_tile(kt, wk_sb, kaug[0], kaug[1], k16, sh)
            for b in range(4):
                nat_unit(b, sh)
        for b in range(4):
            nat_finish(b)

        # ---- main loop over q tiles ---------------------------------------
        for t in range(NT):
            tsl = slice(t * 512, (t + 1) * 512)
            psAs = [ps_av.tile([65, 512], f32, tag=f"pav{h}", name=f"pav{h}")
                    for h in range(2)]
            es = {}

            # extra PE work streamed into this tile's steps: q proj for t+1,
            # then nat pass for t+1 block-major (finish right after a block's
            # 8 slices). Tile 0 additionally carries the v projection, with
            # vproj units placed ahead of the av deadlines that consume them.
            extras = []
            if t + 1 < NT:
                extras.append(("qproj", t + 1))
                for b in range(4):
                    for sh in range(NSH):
                        extras.append(("nat", (t + 1) * 4 + b, sh))
                    extras.append(("fin", (t + 1) * 4 + b))
            if t == 0:
                # av(pair p) at step p+1 consumes v chunks 2p..2p+1 =>
                # vproj tau (chunks 4tau..4tau+3) must precede step 2tau+1.
                # With consumption rate n/16 per step, insert greedily.
                vp_positions = [0, 1, 3, 8, 13, 18, 23, 28]
                for i, pos in enumerate(vp_positions):
                    extras.insert(min(pos, len(extras)), ("vproj", i))
            n_extra = len(extras)

            ei = 0

            def run_extras(upto):
                nonlocal ei
                while ei < min(upto, n_extra):
                    kind, *args = extras[ei]
                    if kind == "qproj":
                        proj_qk_tile(qt, wq_sb, qaug[0], qaug[1], q16, args[0])
                    elif kind == "nat":
                        nat_unit(args[0], args[1])
                    elif kind == "vproj":
                        proj_v_tile(args[0])
                    else:
                        nat_finish(args[0])
                    ei += 1

            for p in range(NP):
                c0 = 2 * p
                for h in range(2):
                    pf = pfine()
                    nc.tensor.matmul(pf[:, 0:512],
                                     kaug[h][:, c0 * 128:(c0 + 1) * 128],
                                     qaug[h][:, tsl],
                                     start=True, stop=True)
                    nc.tensor.matmul(pf[:, 512:1024],
                                     kaug[h][:, (c0 + 1) * 128:(c0 + 2) * 128],
                                     qaug[h][:, tsl],
                                     start=True, stop=True)
                    if p > 0:
                        ep = es.pop((h, p - 1))
                        nc.tensor.matmul(psAs[h][:], v_sb[:, c0 - 2, h, :],
                                         ep[:, 0:512],
                                         start=(p == 1), stop=False,
                                         skip_group_check=True)
                        nc.tensor.matmul(psAs[h][:], v_sb[:, c0 - 1, h, :],
                                         ep[:, 512:1024],
                                         start=False, stop=False,
                                         skip_group_check=True)
                    e = epool.tile([128, 1024], fp16, tag="e", name="e")
                    nc.scalar.activation(e[:], pf[:], Exp)
                    es[(h, p)] = e
                # stream tile t+1 work between fine/av steps
                run_extras((p + 1) * n_extra // NP)

            # tail: av for the last pair
            for h in range(2):
                ep = es.pop((h, NP - 1))
                nc.tensor.matmul(psAs[h][:], v_sb[:, NCH - 2, h, :],
                                 ep[:, 0:512],
                                 start=False, stop=False, skip_group_check=True)
                nc.tensor.matmul(psAs[h][:], v_sb[:, NCH - 1, h, :],
                                 ep[:, 512:1024],
                                 start=False, stop=True, skip_group_check=True)

            # normalize: concat[h] = att@V / rowsums
            for h in range(2):
                psA = psAs[h]
                sums = smalls.tile([1, 512], f32, tag="sums", name="sums")
                nc.vector.tensor_copy(sums[:], psA[64:65, :])
                rec = smalls.tile([1, 512], f32, tag="rec", name="rec")
                nc.vector.reciprocal_approx_fast(rec[:], sums[:])
                rec_r = smalls.tile([1, 512], f32r, tag="rec_r", name="rec_r")
                nc.vector.tensor_copy(rec_r[:], rec[:])
                pr = pfine()
                nc.tensor.matmul(pr[0:64, 0:512], ones64[:], rec_r[:],
                                 start=True, stop=True, skip_group_check=True)
                reps = smalls.tile([64, 512], f32, tag="reps", name="reps")
                nc.vector.tensor_copy(reps[:], pr[0:64, 0:512])
                nc.vector.tensor_mul(concat[h * 64:(h + 1) * 64, tsl],
                                     psA[0:64, :], reps[:])

            # W_O on this q tile
            for b in range(4):
                qb = t * 4 + b
                po = pfine()
                for n in range(2):
                    nc.tensor.matmul(po[:, n * 512:(n + 1) * 512],
                                     concat[:, qb * 128:(qb + 1) * 128],
                                     wo_sb[:, n * 512:(n + 1) * 512],
                                     start=True, stop=True)
                ot = outp.tile([128, 1024], f32, tag="ot", name="ot")
                if b % 2 == 0:
                    nc.scalar.copy(ot[:], po[:])
                else:
                    nc.vector.tensor_copy(ot[:], po[:])
                nc.sync.dma_start(out[qb * 128:(qb + 1) * 128, :], ot[:])

    nc.compile()
    return nc


def _prep_inputs(Q, K, V, W_Q, W_K, W_V, W_O):
    Q = np.ascontiguousarray(np.asarray(Q, dtype=np.float32))
    K = np.ascontiguousarray(np.asarray(K, dtype=np.float32))
    V = np.ascontiguousarray(np.asarray(V, dtype=np.float32))
    W_Q = np.asarray(W_Q, dtype=np.float32)
    W_K = np.asarray(W_K, dtype=np.float32)
    W_V = np.asarray(W_V, dtype=np.float32)
    W_O = np.asarray(W_O, dtype=np.float32)

    QT = np.ascontiguousarray(Q.T)
    KT = np.ascontiguousarray(K.T)
    VT = np.ascontiguousarray(V.T)
    scale = np.float32(0.125)  # 1/sqrt(64), exact power of two

    in_maps = []
    for c in range(NCORES):
        hA, hB = 2 * c, 2 * c + 1
        in_maps.append({
            "qt": QT,
            "kt": KT,
            "vt": VT,
            "wq": np.ascontiguousarray(np.concatenate([W_Q[hA], W_Q[hB]], axis=1)),
            "wk": np.ascontiguousarray(
                np.concatenate([W_K[hA] * scale, W_K[hB] * scale], axis=1)),
            "wv": np.ascontiguousarray(np.concatenate([W_V[hA], W_V[hB]], axis=1)),
            "wo": np.ascontiguousarray(W_O[c * 128:(c + 1) * 128, :]),
        })
    return in_maps


def kernel(Q, K, V, W_Q, W_K, W_V, W_O):
    global LAST_RESULT
    from concourse.bass_utils import run_bass_kernel_spmd

    S = np.asarray(Q).shape[0]
    nc = _cache.get(S)
    if nc is None:
        nc = _build(S)
        _cache[S] = nc

    in_maps = _prep_inputs(Q, K, V, W_Q, W_K, W_V, W_O)
    res = run_bass_kernel_spmd(nc, in_maps, list(range(NCORES)))
    LAST_RESULT = res
    parts = np.stack([res.results[i]["out"] for i in range(NCORES)])
    return parts.sum(axis=0, dtype=np.float32)


# revision 17
# speedup vs baseline: 1.0183x; 1.0183x over previous
"""Trainium2 Bass kernel: 16-head attention (SEQ=4096, D_MODEL=1024, D_K=64).

Sharding: tensor-parallel over heads. 2 heads per core x 8 cores.
W_O is row-sharded; each core returns a partial [S, D] output projection,
summed on the host (the all-reduce of the output projection).

Per-core dataflow:
  Projections of transposed inputs (QT/KT f32r, VT fp16 in DRAM).
  Fine pass (transposed scores [s,q]) runs with an fp16 MOVING operand for
  full PE rate (f32r moving streams at half rate), with accuracy recovered
  by error feedback: the stationary kaug2 [128,128] f32r holds k rows 0-63,
  ones row 64, and k dims 0-62 again in rows 65-127; the moving qaug2
  [128,512] fp16 holds fp16(q), -max, and the fp16 quantization residual
  dq = q - fp16(q) for dims 0-62. The matmul computes
  k.q16 + k[0:63].dq[0:63] - max ~= k.q - max at fp16 speed.
  Natural pass (scores [q,s], fp16, 2 heads packed concurrently on PE row
  groups) feeds per-head DVE max reduces; maxes transposed via a tiny
  matmul against -I into qaug2 row 64.
  One ScalarE exp per [128,1024] PSUM pair -> E fp16; [v|1] @ E accumulates
  att@V + row sums; reciprocal + ones-replication matmul normalizes; W_O
  (fp16) on normalized concat; partial out DMA'd back.

Pipelining: tile t streams tile t+1's q-projection and natural pass between
its fine/av steps (tile 0 also streams the v projection); PSUM banks:
av 2 + fine-pairs 4 + nat 2.
"""

import os
import sys

import numpy as np

for _p in (
    "/root/.axon_site",
    "/root/.axon_site/_ro/trn_rl_repo",
    "/root/.axon_site/_ro/pypackages",
    "/opt/trn_rl_repo",
    "/opt/pypackages",
):
    if os.path.isdir(_p) and _p not in sys.path:
        sys.path.append(_p)

D = 1024
NHEADS = 16
DK = 64
NCORES = 8
S_FULL = 4096

_cache = {}
LAST_RESULT = None  # BassKernelResults of the most recent run (for test harness)


def _build(S):
    import concourse.bass as bass  # noqa: F401
    import concourse.tile as tile
    from concourse import bacc, mybir
    from concourse.masks import make_identity
    from contextlib import ExitStack

    f32 = mybir.dt.float32
    f32r = mybir.dt.float32r
    fp16 = mybir.dt.float16
    X = mybir.AxisListType.X
    Exp = mybir.ActivationFunctionType.Exp

    NT = S // 512   # 512-wide q tiles
    NCH = S // 128  # 128-wide s chunks
    NP = NCH // 2   # pairs of s chunks per tile
    ND = D // 128   # contraction chunks
    NSH = S // 512  # 512-wide s slices for the natural pass

    nc = bacc.Bacc(
        "TRN2",
        target_bir_lowering=False,
        debug=False,
        num_devices=NCORES,
    )
    qt = nc.dram_tensor("qt", [D, S], f32r, kind="ExternalInput")
    kt = nc.dram_tensor("kt", [D, S], f32r, kind="ExternalInput")
    vt = nc.dram_tensor("vt", [D, S], fp16, kind="ExternalInput")
    wq = nc.dram_tensor("wq", [D, 128], f32r, kind="ExternalInput")
    wk = nc.dram_tensor("wk", [D, 128], f32r, kind="ExternalInput")
    wv = nc.dram_tensor("wv", [D, 128], fp16, kind="ExternalInput")
    wo = nc.dram_tensor("wo", [128, D], fp16, kind="ExternalInput")
    out = nc.dram_tensor("out", [S, D], f32, kind="ExternalOutput")

    with tile.TileContext(nc) as tc, ExitStack() as ctx:
        consts = ctx.enter_context(tc.tile_pool(name="consts", bufs=1))
        big = ctx.enter_context(tc.tile_pool(name="big", bufs=1))
        ldpool = ctx.enter_context(tc.tile_pool(name="ld", bufs=3))
        epool = ctx.enter_context(tc.tile_pool(name="e", bufs=3))
        smalls = ctx.enter_context(tc.tile_pool(name="smalls", bufs=4))
        outp = ctx.enter_context(tc.tile_pool(name="outp", bufs=2))
        ps_nat = ctx.enter_context(tc.tile_pool(name="ps_nat", bufs=1, space="PSUM"))
        ps_av = ctx.enter_context(tc.tile_pool(name="ps_av", bufs=1, space="PSUM"))
        ps_pf = ctx.enter_context(tc.tile_pool(name="ps_pf", bufs=2, space="PSUM"))

        def pfine():
            # rotating 2-bank [128, 1024] fp32 tile: fine pairs, W_O, q-proj,
            # k-proj, v-proj, maxT, reps all share this rotation
            return ps_pf.tile([128, 1024], f32, tag="pf", name="pf")

        def pnat():
            return ps_nat.tile([128, 1024], f32, tag="pn", name="pn")

        # constants
        ident_f = consts.tile([128, 128], f32)
        make_identity(nc, ident_f)
        identn = consts.tile([128, 128], fp16)  # -I for the max transpose
        nc.vector.tensor_scalar_mul(identn[:], ident_f[:], -1.0)
        ones64 = consts.tile([1, 64], f32r)
        nc.vector.memset(ones64[:].bitcast(f32), 1.0)

        # weights
        wq_sb = consts.tile([128, ND, 128], f32r)
        nc.sync.dma_start(wq_sb[:], wq.rearrange("(o p) f -> p o f", p=128))
        wk_sb = consts.tile([128, ND, 128], f32r)
        nc.sync.dma_start(wk_sb[:], wk.rearrange("(o p) f -> p o f", p=128))
        wv_sb = consts.tile([128, ND, 128], fp16)
        nc.sync.dma_start(wv_sb[:], wv.rearrange("(o p) f -> p o f", p=128))
        wo_sb = consts.tile([128, D], fp16)
        nc.sync.dma_start(wo_sb[:], wo[:])

        # big SBUF tensors -- all-fp16 fine operands with split error feedback
        # (DVE partition bases must be 32-aligned; row 64 of the 64-95 group
        # is written as part of the group then clobbered by ones/-max):
        # kaug2[h] fp16 stationary: rows 0-63 k16; row 64 ones; rows 65-95
        #   k16 dims 1-31; rows 96-127 dk = fp16(k - k16) dims 0-31.
        # qaug2[h] fp16 moving: rows 0-63 q16; row 64 -max; rows 65-95
        #   dq = fp16(q - q16) dims 1-31; rows 96-127 q16 dims 0-31.
        # => scores = k16.q16 + k16[1:32].dq[1:32] + dk[0:32].q16[0:32] - max
        qaug = [big.tile([128, S], fp16, tag=f"qaug{h}", name=f"qaug{h}") for h in range(2)]
        kaug = [big.tile([128, S], fp16, tag=f"kaug{h}", name=f"kaug{h}") for h in range(2)]
        v_sb = big.tile([128, NCH, 2, 65], fp16, tag="v", name="v_sb")
        q16 = big.tile([128, S], fp16, tag="q16", name="q16")
        k16 = big.tile([128, S], fp16, tag="k16", name="k16")
        concat = big.tile([128, S], fp16, tag="concat", name="concat")
        nc.vector.memset(v_sb[:, :, :, 64:65], 1.0)

        # per-block running maxes for the natural pass: [128, 2 heads, NSH]
        mx = [smalls.tile([128, 2, NSH], f32, tag=f"mx{b}", name=f"mx{b}")
              for b in range(4)]

        # ---- projection helpers -------------------------------------------
        def proj_q_tile(t):
            """Project one 512-col slice of q; fill qaug2 (q16/dq) + q16."""
            tsl = slice(t * 512, (t + 1) * 512)
            ps = pfine()
            for d in range(ND):
                lt = ldpool.tile([128, 512], f32r, tag="ld", name="ld")
                nc.sync.dma_start(lt[:], qt[d * 128:(d + 1) * 128, tsl])
                nc.tensor.matmul(ps[:, 0:512], wq_sb[:, d, :], lt[:],
                                 start=(d == 0), stop=(d == ND - 1))
            for h in range(2):
                hp = slice(h * 64, h * 64 + 64)
                nc.scalar.copy(qaug[h][0:64, tsl], ps[hp, 0:512])
            for h in range(2):
                h32 = slice(h * 64, h * 64 + 32)
                # dq dims 0-31 into rows 64-95 (row 64 clobbered by -max later)
                nc.vector.tensor_sub(qaug[h][64:96, tsl], ps[h32, 0:512],
                                     qaug[h][0:32, tsl])
                # q16 dims 0-31 again into rows 96-127 (for the dk residual)
                nc.scalar.copy(qaug[h][96:128, tsl], ps[h32, 0:512])
            nc.vector.tensor_copy(q16[:, tsl], ps[:, 0:512])

        def proj_k_tile(t):
            """Project one 512-col slice of k; fill kaug2 (k/ones/kres) + k16."""
            tsl = slice(t * 512, (t + 1) * 512)
            ps = pfine()
            for d in range(ND):
                lt = ldpool.tile([128, 512], f32r, tag="ld", name="ld")
                nc.sync.dma_start(lt[:], kt[d * 128:(d + 1) * 128, tsl])
                nc.tensor.matmul(ps[:, 0:512], wk_sb[:, d, :], lt[:],
                                 start=(d == 0), stop=(d == ND - 1))
            for h in range(2):
                hp = slice(h * 64, h * 64 + 64)
                nc.scalar.copy(kaug[h][0:64, tsl], ps[hp, 0:512])
            for h in range(2):
                h32 = slice(h * 64, h * 64 + 32)
                # dk dims 0-31 into rows 96-127
                nc.vector.tensor_sub(kaug[h][96:128, tsl], ps[h32, 0:512],
                                     kaug[h][0:32, tsl])
                # k16 dims 0-31 into rows 64-95 (row 64 re-memset to 1 after)
                nc.scalar.copy(kaug[h][64:96, tsl], ps[h32, 0:512])
            nc.vector.tensor_copy(k16[:, tsl], ps[:, 0:512])

        def proj_v_tile(t):
            vts = ldpool.tile([128, ND, 512], fp16, tag="vts", name="vts", bufs=2)
            nc.sync.dma_start(
                vts[:], vt[:, t * 512:(t + 1) * 512].rearrange("(o p) s -> p o s", p=128))
            ps = pfine()
            for sc in range(4):
                for d in range(ND):
                    nc.tensor.matmul(ps[:, sc * 128:(sc + 1) * 128],
                                     vts[:, d, sc * 128:(sc + 1) * 128],
                                     wv_sb[:, d, :],
                                     start=(d == 0), stop=(d == ND - 1),
                                     skip_group_check=True)
            for sc in range(4):
                c = t * 4 + sc
                nc.vector.tensor_copy(
                    v_sb[:, c, :, 0:64],
                    ps[:, sc * 128:(sc + 1) * 128].rearrange("p (h f) -> p h f", h=2))

        # ---- natural pass units -------------------------------------------
        def nat_unit(b, sh):
            """Natural scores for q-block b (global 128-block idx), s-slice sh.
            Both heads packed concurrently on PE row groups (0,0)/(64,0)."""
            qsl = slice(b * 128, (b + 1) * 128)
            ssl = slice(sh * 512, (sh + 1) * 512)
            pn = pnat()
            nc.tensor.matmul(pn[:, 0:512], q16[0:64, qsl], k16[0:64, ssl],
                             start=True, stop=True)
            nc.tensor.matmul(pn[:, 512:1024], q16[64:128, qsl], k16[64:128, ssl],
                             start=True, stop=True)
            for h in range(2):
                nc.vector.reduce_max(mx[b % 4][:, h, sh:sh + 1],
                                     pn[:, h * 512:(h + 1) * 512], axis=X)

        def nat_finish(b):
            """Final per-head max for q-block b -> -max into qaug2 row 64."""
            qsl = slice(b * 128, (b + 1) * 128)
            m2 = smalls.tile([128, 2, 1], fp16, tag="m2", name="m2")
            nc.vector.reduce_max(m2[:], mx[b % 4][:], axis=X)
            pt = pfine()
            for h in range(2):
                nc.tensor.matmul(pt[0:1, h * 128:(h + 1) * 128],
                                 m2[:, h, :], identn[:],
                                 start=True, stop=True, skip_group_check=True)
            for h in range(2):
                nc.vector.tensor_copy(qaug[h][64:65, qsl],
                                      pt[0:1, h * 128:(h + 1) * 128])

        # ---- prologue -----------------------------------------------------
        # q proj for tile 0 first (small), then k streamed with tile-0 nat
        # units right behind each 512-slice. v-projection is deferred into
        # tile 0's streamed extras so its DMA overlaps attention compute.
        proj_q_tile(0)
        for sh in range(NSH):
            proj_k_tile(sh)
            for b in range(4):
                nat_unit(b, sh)
        # ones row of kaug2 (clobbers the copied k16 dim-0 row)
        for h in range(2):
            nc.vector.memset(kaug[h][64:65, :], 1.0)
        for b in range(4):
            nat_finish(b)

        # ---- main loop over q tiles ---------------------------------------
        for t in range(NT):
            tsl = slice(t * 512, (t + 1) * 512)
            psAs = [ps_av.tile([65, 512], f32, tag=f"pav{h}", name=f"pav{h}")
                    for h in range(2)]
            es = {}

            extras = []
            if t + 1 < NT:
                extras.append(("qproj", t + 1))
                for b in range(4):
                    for sh in range(NSH):
                        extras.append(("nat", (t + 1) * 4 + b, sh))
                    extras.append(("fin", (t + 1) * 4 + b))
            if t == 0:
                # av(pair p) at step p+1 consumes v chunks 2p..2p+1 =>
                # vproj tau (chunks 4tau..4tau+3) must precede step 2tau+1.
                vp_positions = [0, 1, 3, 8, 13, 18, 23, 28]
                for i, pos in enumerate(vp_positions):
                    extras.insert(min(pos, len(extras)), ("vproj", i))
            n_extra = len(extras)

            ei = 0

            def run_extras(upto):
                nonlocal ei
                while ei < min(upto, n_extra):
                    kind, *args = extras[ei]
                    if kind == "qproj":
                        proj_q_tile(args[0])
                    elif kind == "nat":
                        nat_unit(args[0], args[1])
                    elif kind == "vproj":
                        proj_v_tile(args[0])
                    else:
                        nat_finish(args[0])
                    ei += 1

            for p in range(NP):
                c0 = 2 * p
                for h in range(2):
                    pf = pfine()
                    nc.tensor.matmul(pf[:, 0:512],
                                     kaug[h][:, c0 * 128:(c0 + 1) * 128],
                                     qaug[h][:, tsl],
                                     start=True, stop=True)
                    nc.tensor.matmul(pf[:, 512:1024],
                                     kaug[h][:, (c0 + 1) * 128:(c0 + 2) * 128],
                                     qaug[h][:, tsl],
                                     start=True, stop=True)
                    if p > 0:
                        ep = es.pop((h, p - 1))
                        nc.tensor.matmul(psAs[h][:], v_sb[:, c0 - 2, h, :],
                                         ep[:, 0:512],
                                         start=(p == 1), stop=False,
                                         skip_group_check=True)
                        nc.tensor.matmul(psAs[h][:], v_sb[:, c0 - 1, h, :],
                                         ep[:, 512:1024],
                                         start=False, stop=False,
                                         skip_group_check=True)
                    e = epool.tile([128, 1024], fp16, tag="e", name="e")
                    nc.scalar.activation(e[:], pf[:], Exp)
                    es[(h, p)] = e
                # stream tile t+1 work between fine/av steps
                run_extras((p + 1) * n_extra // NP)

            # tail: av for the last pair
            for h in range(2):
                ep = es.pop((h, NP - 1))
                nc.tensor.matmul(psAs[h][:], v_sb[:, NCH - 2, h, :],
                                 ep[:, 0:512],
                                 start=False, stop=False, skip_group_check=True)
                nc.tensor.matmul(psAs[h][:], v_sb[:, NCH - 1, h, :],
                                 ep[:, 512:1024],
                                 start=False, stop=True, skip_group_check=True)

            # normalize: concat[h] = att@V / rowsums
            for h in range(2):
                psA = psAs[h]
                sums = smalls.tile([1, 512], f32, tag="sums", name="sums")
                nc.vector.tensor_copy(sums[:], psA[64:65, :])
                rec = smalls.tile([1, 512], f32, tag="rec", name="rec")
                nc.vector.reciprocal_approx_fast(rec[:], sums[:])
                rec_r = smalls.tile([1, 512], f32r, tag="rec_r", name="rec_r")
                nc.vector.tensor_copy(rec_r[:], rec[:])
                pr = pfine()
                nc.tensor.matmul(pr[0:64, 0:512], ones64[:], rec_r[:],
                                 start=True, stop=True, skip_group_check=True)
                reps = smalls.tile([64, 512], fp16, tag="reps", name="reps")
                nc.vector.tensor_copy(reps[:], pr[0:64, 0:512])
                nc.vector.tensor_mul(concat[h * 64:(h + 1) * 64, tsl],
                                     psA[0:64, :], reps[:])

            # W_O on this q tile
            for b in range(4):
                qb = t * 4 + b
                po = pfine()
                for n in range(2):
                    nc.tensor.matmul(po[:, n * 512:(n + 1) * 512],
                                     concat[:, qb * 128:(qb + 1) * 128],
                                     wo_sb[:, n * 512:(n + 1) * 512],
                                     start=True, stop=True)
                ot = outp.tile([128, 1024], f32, tag="ot", name="ot")
                nc.scalar.copy(ot[:], po[:])
                nc.sync.dma_start(out[qb * 128:(qb + 1) * 128, :], ot[:])

    nc.compile()
    return nc


def _prep_inputs(Q, K, V, W_Q, W_K, W_V, W_O):
    Q = np.ascontiguousarray(np.asarray(Q, dtype=np.float32))
    K = np.ascontiguousarray(np.asarray(K, dtype=np.float32))
    V = np.ascontiguousarray(np.asarray(V, dtype=np.float32))
    W_Q = np.asarray(W_Q, dtype=np.float32)
    W_K = np.asarray(W_K, dtype=np.float32)
    W_V = np.asarray(W_V, dtype=np.float32)
    W_O = np.asarray(W_O, dtype=np.float32)

    QT = np.ascontiguousarray(Q.T)
    KT = np.ascontiguousarray(K.T)
    VT = np.ascontiguousarray(V.T.astype(np.float16))
    scale = np.float32(0.125)  # 1/sqrt(64), exact power of two

    in_maps = []
    for c in range(NCORES):
        hA, hB = 2 * c, 2 * c + 1
        in_maps.append({
            "qt": QT,
            "kt": KT,
            "vt": VT,
            "wq": np.ascontiguousarray(np.concatenate([W_Q[hA], W_Q[hB]], axis=1)),
            "wk": np.ascontiguousarray(
                np.concatenate([W_K[hA] * scale, W_K[hB] * scale], axis=1)),
            "wv": np.ascontiguousarray(
                np.concatenate([W_V[hA], W_V[hB]], axis=1).astype(np.float16)),
            "wo": np.ascontiguousarray(W_O[c * 128:(c + 1) * 128, :].astype(np.float16)),
        })
    return in_maps


def kernel(Q, K, V, W_Q, W_K, W_V, W_O):
    global LAST_RESULT
    from concourse.bass_utils import run_bass_kernel_spmd

    S = np.asarray(Q).shape[0]
    nc = _cache.get(S)
    if nc is None:
        nc = _build(S)
        _cache[S] = nc

    in_maps = _prep_inputs(Q, K, V, W_Q, W_K, W_V, W_O)
    res = run_bass_kernel_spmd(nc, in_maps, list(range(NCORES)))
    LAST_RESULT = res
    parts = np.stack([res.results[i]["out"] for i in range(NCORES)])
    return parts.sum(axis=0, dtype=np.float32)


# revision 19
# speedup vs baseline: 1.0967x; 1.0770x over previous
"""Trainium2 Bass kernel: 16-head attention (SEQ=4096, D_MODEL=1024, D_K=64).

Sharding: tensor-parallel over heads. 2 heads per core x 8 cores.
W_O is row-sharded; each core returns a partial [S, D] output projection,
summed on the host (the all-reduce of the output projection).

Per-core dataflow:
  Projections of transposed inputs (QT/KT f32r, VT fp16 in DRAM).
  Fine pass (transposed scores [s,q]) runs with an fp16 MOVING operand for
  full PE rate (f32r moving streams at half rate), with accuracy recovered
  by error feedback: the stationary kaug2 [128,128] f32r holds k rows 0-63,
  ones row 64, and k dims 0-62 again in rows 65-127; the moving qaug2
  [128,512] fp16 holds fp16(q), -max, and the fp16 quantization residual
  dq = q - fp16(q) for dims 0-62. The matmul computes
  k.q16 + k[0:63].dq[0:63] - max ~= k.q - max at fp16 speed.
  Natural pass (scores [q,s], fp16, 2 heads packed concurrently on PE row
  groups) feeds per-head DVE max reduces; maxes transposed via a tiny
  matmul against -I into qaug2 row 64.
  One ScalarE exp per [128,1024] PSUM pair -> E fp16; [v|1] @ E accumulates
  att@V + row sums; reciprocal + ones-replication matmul normalizes; W_O
  (fp16) on normalized concat; partial out DMA'd back.

Pipelining: tile t streams tile t+1's q-projection and natural pass between
its fine/av steps (tile 0 also streams the v projection); PSUM banks:
av 2 + fine-pairs 4 + nat 2.
"""

import os
import sys

import numpy as np

for _p in (
    "/root/.axon_site",
    "/root/.axon_site/_ro/trn_rl_repo",
    "/root/.axon_site/_ro/pypackages",
    "/opt/trn_rl_repo",
    "/opt/pypackages",
):
    if os.path.isdir(_p) and _p not in sys.path:
        sys.path.append(_p)

D = 1024
NHEADS = 16
DK = 64
NCORES = 8
S_FULL = 4096

_cache = {}
LAST_RESULT = None  # BassKernelResults of the most recent run (for test harness)


def _build(S):
    import concourse.bass as bass  # noqa: F401
    import concourse.tile as tile
    from concourse import bacc, mybir
    from concourse.masks import make_identity
    from contextlib import ExitStack

    f32 = mybir.dt.float32
    f32r = mybir.dt.float32r
    fp16 = mybir.dt.float16
    X = mybir.AxisListType.X
    Exp = mybir.ActivationFunctionType.Exp

    NT = S // 512   # 512-wide q tiles
    NCH = S // 128  # 128-wide s chunks
    NP = NCH // 2   # pairs of s chunks per tile
    ND = D // 128   # contraction chunks
    NSH = S // 512  # 512-wide s slices for the natural pass

    nc = bacc.Bacc(
        "TRN2",
        target_bir_lowering=False,
        debug=False,
        num_devices=NCORES,
    )
    qt = nc.dram_tensor("qt", [D, S], f32r, kind="ExternalInput")
    kt = nc.dram_tensor("kt", [D, S], f32r, kind="ExternalInput")
    vt = nc.dram_tensor("vt", [D, S], fp16, kind="ExternalInput")
    wq = nc.dram_tensor("wq", [D, 128], f32r, kind="ExternalInput")
    wk = nc.dram_tensor("wk", [D, 128], f32r, kind="ExternalInput")
    wv = nc.dram_tensor("wv", [D, 128], fp16, kind="ExternalInput")
    wo = nc.dram_tensor("wo", [128, D], fp16, kind="ExternalInput")
    out = nc.dram_tensor("out", [S, D], f32, kind="ExternalOutput")

    with tile.TileContext(nc) as tc, ExitStack() as ctx:
        consts = ctx.enter_context(tc.tile_pool(name="consts", bufs=1))
        big = ctx.enter_context(tc.tile_pool(name="big", bufs=1))
        ldpool = ctx.enter_context(tc.tile_pool(name="ld", bufs=3))
        epool = ctx.enter_context(tc.tile_pool(name="e", bufs=5))
        smalls = ctx.enter_context(tc.tile_pool(name="smalls", bufs=4))
        outp = ctx.enter_context(tc.tile_pool(name="outp", bufs=2))
        ps_nat = ctx.enter_context(tc.tile_pool(name="ps_nat", bufs=1, space="PSUM"))
        ps_av = ctx.enter_context(tc.tile_pool(name="ps_av", bufs=1, space="PSUM"))
        ps_pf = ctx.enter_context(tc.tile_pool(name="ps_pf", bufs=2, space="PSUM"))

        def pfine():
            # private 2-bank [128, 1024] fp32 rotation: fine pairs ONLY, so
            # the fine->exp->av chain never couples to other psum users
            return ps_pf.tile([128, 1024], f32, tag="pf", name="pf")

        def pmisc():
            # shared 2-bank rotation: nat units, q/k/v proj, W_O, maxT, reps
            return ps_nat.tile([128, 1024], f32, tag="pn", name="pn")

        # constants
        ident_f = consts.tile([128, 128], f32)
        make_identity(nc, ident_f)
        identn = consts.tile([128, 128], fp16)  # -I for the max transpose
        nc.vector.tensor_scalar_mul(identn[:], ident_f[:], -1.0)
        ones64 = consts.tile([1, 64], f32r)
        nc.vector.memset(ones64[:].bitcast(f32), 1.0)

        # weights
        wq_sb = consts.tile([128, ND, 128], f32r)
        nc.sync.dma_start(wq_sb[:], wq.rearrange("(o p) f -> p o f", p=128))
        wk_sb = consts.tile([128, ND, 128], f32r)
        nc.sync.dma_start(wk_sb[:], wk.rearrange("(o p) f -> p o f", p=128))
        wv_sb = consts.tile([128, ND, 128], fp16)
        nc.sync.dma_start(wv_sb[:], wv.rearrange("(o p) f -> p o f", p=128))
        wo_sb = consts.tile([128, D], fp16)
        nc.sync.dma_start(wo_sb[:], wo[:])

        # big SBUF tensors -- all-fp16 fine operands with split error feedback
        # (DVE partition bases must be 32-aligned; row 64 of the 64-95 group
        # is written as part of the group then clobbered by ones/-max):
        # kaug2[h] fp16 stationary: rows 0-63 k16; row 64 ones; rows 65-95
        #   k16 dims 1-31; rows 96-127 dk = fp16(k - k16) dims 0-31.
        # qaug2[h] fp16 moving: rows 0-63 q16; row 64 -max; rows 65-95
        #   dq = fp16(q - q16) dims 1-31; rows 96-127 q16 dims 0-31.
        # => scores = k16.q16 + k16[1:32].dq[1:32] + dk[0:32].q16[0:32] - max
        qaug = [big.tile([128, S], fp16, tag=f"qaug{h}", name=f"qaug{h}") for h in range(2)]
        kaug = [big.tile([128, S], fp16, tag=f"kaug{h}", name=f"kaug{h}") for h in range(2)]
        v_sb = big.tile([128, NCH, 2, 65], fp16, tag="v", name="v_sb")
        q16 = big.tile([128, S], fp16, tag="q16", name="q16")
        k16 = big.tile([128, S], fp16, tag="k16", name="k16")
        concat = big.tile([128, S], fp16, tag="concat", name="concat")
        nc.vector.memset(v_sb[:, :, :, 64:65], 1.0)

        # per-block running maxes for the natural pass: [128, 2 heads, NSP]
        NSP = NSH // 2
        mx = [smalls.tile([128, 2, NSP], f32, tag=f"mx{b}", name=f"mx{b}")
              for b in range(4)]

        # ---- projection helpers -------------------------------------------
        def proj_q_tile(t):
            """Project one 512-col slice of q; fill qaug2 (q16/dq) + q16."""
            tsl = slice(t * 512, (t + 1) * 512)
            ps = pmisc()
            for d in range(ND):
                lt = ldpool.tile([128, 512], f32r, tag="ld", name="ld")
                nc.sync.dma_start(lt[:], qt[d * 128:(d + 1) * 128, tsl])
                nc.tensor.matmul(ps[:, 0:512], wq_sb[:, d, :], lt[:],
                                 start=(d == 0), stop=(d == ND - 1))
            for h in range(2):
                hp = slice(h * 64, h * 64 + 64)
                nc.scalar.copy(qaug[h][0:64, tsl], ps[hp, 0:512])
            for h in range(2):
                h32 = slice(h * 64, h * 64 + 32)
                # dq dims 0-31 into rows 64-95 (row 64 clobbered by -max later)
                nc.vector.tensor_sub(qaug[h][64:96, tsl], ps[h32, 0:512],
                                     qaug[h][0:32, tsl])
                # q16 dims 0-31 again into rows 96-127 (for the dk residual)
                nc.vector.tensor_copy(qaug[h][96:128, tsl], ps[h32, 0:512])
            nc.vector.tensor_copy(q16[:, tsl], ps[:, 0:512])

        def proj_k_tile(t):
            """Project one 512-col slice of k; fill kaug2 (k/ones/kres) + k16."""
            tsl = slice(t * 512, (t + 1) * 512)
            ps = pmisc()
            for d in range(ND):
                lt = ldpool.tile([128, 512], f32r, tag="ld", name="ld")
                nc.sync.dma_start(lt[:], kt[d * 128:(d + 1) * 128, tsl])
                nc.tensor.matmul(ps[:, 0:512], wk_sb[:, d, :], lt[:],
                                 start=(d == 0), stop=(d == ND - 1))
            for h in range(2):
                hp = slice(h * 64, h * 64 + 64)
                nc.scalar.copy(kaug[h][0:64, tsl], ps[hp, 0:512])
            for h in range(2):
                h32 = slice(h * 64, h * 64 + 32)
                # dk dims 0-31 into rows 96-127
                nc.vector.tensor_sub(kaug[h][96:128, tsl], ps[h32, 0:512],
                                     kaug[h][0:32, tsl])
                # k16 dims 0-31 into rows 64-95 (row 64 re-memset to 1 after)
                nc.scalar.copy(kaug[h][64:96, tsl], ps[h32, 0:512])
            nc.vector.tensor_copy(k16[:, tsl], ps[:, 0:512])

        def proj_v_tile(t):
            vts = ldpool.tile([128, ND, 512], fp16, tag="vts", name="vts", bufs=2)
            nc.sync.dma_start(
                vts[:], vt[:, t * 512:(t + 1) * 512].rearrange("(o p) s -> p o s", p=128))
            ps = pmisc()
            for sc in range(4):
                for d in range(ND):
                    nc.tensor.matmul(ps[:, sc * 128:(sc + 1) * 128],
                                     vts[:, d, sc * 128:(sc + 1) * 128],
                                     wv_sb[:, d, :],
                                     start=(d == 0), stop=(d == ND - 1),
                                     skip_group_check=True)
            for sc in range(4):
                c = t * 4 + sc
                nc.vector.tensor_copy(
                    v_sb[:, c, :, 0:64],
                    ps[:, sc * 128:(sc + 1) * 128].rearrange("p (h f) -> p h f", h=2))

        # ---- natural pass units -------------------------------------------
        def nat_unit(b, h, sp):
            """Natural scores for q-block b (global idx), head h, s-slice
            pair sp (two 512-wide slices into one 2-bank tile, then one flat
            [128,1024] DVE max reduce)."""
            qsl = slice(b * 128, (b + 1) * 128)
            hp = slice(h * 64, h * 64 + 64)
            pn = pmisc()
            nc.tensor.matmul(pn[:, 0:512], q16[hp, qsl],
                             k16[hp, (2 * sp) * 512:(2 * sp + 1) * 512],
                             start=True, stop=True)
            nc.tensor.matmul(pn[:, 512:1024], q16[hp, qsl],
                             k16[hp, (2 * sp + 1) * 512:(2 * sp + 2) * 512],
                             start=True, stop=True)
            nc.vector.reduce_max(mx[b % 4][:, h, sp:sp + 1], pn[:], axis=X)

        def nat_finish(b):
            """Final per-head max for q-block b -> -max into qaug2 row 64."""
            qsl = slice(b * 128, (b + 1) * 128)
            m2 = smalls.tile([128, 2, 1], fp16, tag="m2", name="m2")
            nc.vector.reduce_max(m2[:], mx[b % 4][:], axis=X)
            pt = pmisc()
            for h in range(2):
                nc.tensor.matmul(pt[0:1, h * 128:(h + 1) * 128],
                                 m2[:, h, :], identn[:],
                                 start=True, stop=True, skip_group_check=True)
            for h in range(2):
                nc.vector.tensor_copy(qaug[h][64:65, qsl],
                                      pt[0:1, h * 128:(h + 1) * 128])

        # ---- prologue -----------------------------------------------------
        # q proj for tile 0 first (small), then k streamed with tile-0 nat
        # units right behind each 512-slice. v-projection is deferred into
        # tile 0's streamed extras so its DMA overlaps attention compute.
        proj_q_tile(0)
        for sp in range(NSH // 2):
            proj_k_tile(2 * sp)
            proj_k_tile(2 * sp + 1)
            for b in range(4):
                for h in range(2):
                    nat_unit(b, h, sp)
        # ones row of kaug2 (clobbers the copied k16 dim-0 row)
        for h in range(2):
            nc.vector.memset(kaug[h][64:65, :], 1.0)
        for b in range(4):
            nat_finish(b)

        # ---- main loop over q tiles ---------------------------------------
        for t in range(NT):
            tsl = slice(t * 512, (t + 1) * 512)
            psAs = [ps_av.tile([65, 512], f32, tag=f"pav{h}", name=f"pav{h}")
                    for h in range(2)]
            es = {}

            extras = []
            if t + 1 < NT:
                extras.append(("qproj", t + 1))
                for b in range(4):
                    for sp in range(NSP):
                        for h in range(2):
                            extras.append(("nat", (t + 1) * 4 + b, h, sp))
                    extras.append(("fin", (t + 1) * 4 + b))
            if t == 0:
                # av(pair p) at step p+1 consumes v chunks 2p..2p+1 =>
                # vproj tau (chunks 4tau..4tau+3) must precede step 2tau+1.
                vp_positions = [0, 1, 3, 8, 13, 18, 23, 28]
                for i, pos in enumerate(vp_positions):
                    extras.insert(min(pos, len(extras)), ("vproj", i))
            n_extra = len(extras)

            ei = 0

            def run_extras(upto):
                nonlocal ei
                while ei < min(upto, n_extra):
                    kind, *args = extras[ei]
                    if kind == "qproj":
                        proj_q_tile(args[0])
                    elif kind == "nat":
                        nat_unit(args[0], args[1], args[2])
                    elif kind == "vproj":
                        proj_v_tile(args[0])
                    else:
                        nat_finish(args[0])
                    ei += 1

            for p in range(NP):
                c0 = 2 * p
                for h in range(2):
                    pf = pfine()
                    nc.tensor.matmul(pf[:, 0:512],
                                     kaug[h][:, c0 * 128:(c0 + 1) * 128],
                                     qaug[h][:, tsl],
                                     start=True, stop=True)
                    nc.tensor.matmul(pf[:, 512:1024],
                                     kaug[h][:, (c0 + 1) * 128:(c0 + 2) * 128],
                                     qaug[h][:, tsl],
                                     start=True, stop=True)
                    e = epool.tile([128, 1024], fp16, tag="e", name="e")
                    nc.scalar.activation(e[:], pf[:], Exp)
                    es[(h, p)] = e
                # av consumes the pair exp'd two steps ago (slack for ScalarE)
                if p >= 2:
                    for h in range(2):
                        ep = es.pop((h, p - 2))
                        cc = 2 * (p - 2)
                        nc.tensor.matmul(psAs[h][:], v_sb[:, cc, h, :],
                                         ep[:, 0:512],
                                         start=(p == 2), stop=False,
                                         skip_group_check=True)
                        nc.tensor.matmul(psAs[h][:], v_sb[:, cc + 1, h, :],
                                         ep[:, 512:1024],
                                         start=False, stop=False,
                                         skip_group_check=True)
                # stream tile t+1 work between fine/av steps
                run_extras((p + 1) * n_extra // NP)

            # tail: av for the last two pairs
            for pp in (NP - 2, NP - 1):
                for h in range(2):
                    ep = es.pop((h, pp))
                    nc.tensor.matmul(psAs[h][:], v_sb[:, 2 * pp, h, :],
                                     ep[:, 0:512],
                                     start=False, stop=False, skip_group_check=True)
                    nc.tensor.matmul(psAs[h][:], v_sb[:, 2 * pp + 1, h, :],
                                     ep[:, 512:1024],
                                     start=False, stop=(pp == NP - 1),
                                     skip_group_check=True)

            # normalize: concat[h] = att@V / rowsums
            for h in range(2):
                psA = psAs[h]
                sums = smalls.tile([1, 512], f32, tag="sums", name="sums")
                nc.scalar.copy(sums[:], psA[64:65, :])
                rec = smalls.tile([1, 512], f32, tag="rec", name="rec")
                nc.vector.reciprocal_approx_fast(rec[:], sums[:])
                rec_r = smalls.tile([1, 512], f32r, tag="rec_r", name="rec_r")
                nc.vector.tensor_copy(rec_r[:], rec[:])
                pr = pmisc()
                nc.tensor.matmul(pr[0:64, 0:512], ones64[:], rec_r[:],
                                 start=True, stop=True, skip_group_check=True)
                reps = smalls.tile([64, 512], fp16, tag="reps", name="reps")
                nc.scalar.copy(reps[:], pr[0:64, 0:512])
                nc.vector.tensor_mul(concat[h * 64:(h + 1) * 64, tsl],
                                     psA[0:64, :], reps[:])

            # W_O on this q tile
            for b in range(4):
                qb = t * 4 + b
                po = pmisc()
                for n in range(2):
                    nc.tensor.matmul(po[:, n * 512:(n + 1) * 512],
                                     concat[:, qb * 128:(qb + 1) * 128],
                                     wo_sb[:, n * 512:(n + 1) * 512],
                                     start=True, stop=True)
                ot = outp.tile([128, 1024], f32, tag="ot", name="ot")
                nc.scalar.copy(ot[:], po[:])
                nc.sync.dma_start(out[qb * 128:(qb + 1) * 128, :], ot[:])

    nc.compile()
    return nc


def _prep_inputs(Q, K, V, W_Q, W_K, W_V, W_O):
    Q = np.ascontiguousarray(np.asarray(Q, dtype=np.float32))
    K = np.ascontiguousarray(np.asarray(K, dtype=np.float32))
    V = np.ascontiguousarray(np.asarray(V, dtype=np.float32))
    W_Q = np.asarray(W_Q, dtype=np.float32)
    W_K = np.asarray(W_K, dtype=np.float32)
    W_V = np.asarray(W_V, dtype=np.float32)
    W_O = np.asarray(W_O, dtype=np.float32)

    QT = np.ascontiguousarray(Q.T)
    KT = np.ascontiguousarray(K.T)
    VT = np.ascontiguousarray(V.T.astype(np.float16))
    scale = np.float32(0.125)  # 1/sqrt(64), exact power of two

    in_maps = []
    for c in range(NCORES):
        hA, hB = 2 * c, 2 * c + 1
        in_maps.append({
            "qt": QT,
            "kt": KT,
            "vt": VT,
            "wq": np.ascontiguousarray(np.concatenate([W_Q[hA], W_Q[hB]], axis=1)),
            "wk": np.ascontiguousarray(
                np.concatenate([W_K[hA] * scale, W_K[hB] * scale], axis=1)),
            "wv": np.ascontiguousarray(
                np.concatenate([W_V[hA], W_V[hB]], axis=1).astype(np.float16)),
            "wo": np.ascontiguousarray(W_O[c * 128:(c + 1) * 128, :].astype(np.float16)),
        })
    return in_maps


def kernel(Q, K, V, W_Q, W_K, W_V, W_O):
    global LAST_RESULT
    from concourse.bass_utils import run_bass_kernel_spmd

    S = np.asarray(Q).shape[0]
    nc = _cache.get(S)
    if nc is None:
        nc = _build(S)
        _cache[S] = nc

    in_maps = _prep_inputs(Q, K, V, W_Q, W_K, W_V, W_O)
    res = run_bass_kernel_spmd(nc, in_maps, list(range(NCORES)))
    LAST_RESULT = res
    parts = np.stack([res.results[i]["out"] for i in range(NCORES)])
    return parts.sum(axis=0, dtype=np.float32)
